# revision 5
# baseline (speedup 1.0000x reference)
"""CppnPotentialCAStep Trainium2 kernel.

Reference computation (B=4, S=96, C=8, P=16, R=2, KS=5):
  x       = input[..., c0_idx]                          [B,S,S,S,P]
  padded  = circular-pad(x, R) in z,y,x
  pot     = depthwise_conv3d(padded, kernels)           [B,S,S,S,P]
  delta   = (exp(-(pot-m)^2/(2 s^2))*2 - 1) * h
  field   = scatter_add(delta -> c1_idx channels)       [B,S,S,S,C]
  out     = clip(input + field/T, 0, 1)

Mapping: shard output z (96) into 8 blocks of 12 (one per NeuronCore).
Host pre-wraps halos into a per-core bf16 slab [16, 100, B, 100, C].
Per core, 24 y-tiles; each loads an SBUF tile with partition=(z=16,y=8)
=128 and free=(b, x=100, c). The (dz,dy) taps live in banded stationary
matrices (two pairs sharing a source channel packed per matmul at
partition offsets 0/64), and the 5 dx taps accumulate in PSUM via
x-shifted moving-operand access patterns. The gate scale k=1/(sqrt2 s)
is folded into the conv weights so ScalarE computes sq=Square(pot*k+b)
straight from PSUM (bf16 out), one batched Exp per tile, then per-pair
fused scale+scatter-add on VectorE into a bf16 channel-major output
accumulator initialized host-side with input + sum(-h/T) per channel.
Clip+cast to fp32 on VectorE, store.

This walrus build encodes at most ONE semaphore wait per instruction
(two for EventSemaphore); the Tile scheduler emits more. `_legalize`
post-processes the BIR JSON, splitting excess waits onto inserted
wait-only EventSemaphore instructions on the same engine queue.
"""

import json
import os
import numpy as np

import concourse.bass as bass
import concourse.mybir as mybir
from concourse.tile import TileContext
from contextlib import ExitStack

try:
    import ml_dtypes
    BF16_NP = ml_dtypes.bfloat16
except Exception:  # pragma: no cover
    BF16_NP = None

F32 = mybir.dt.float32
BF16 = mybir.dt.bfloat16

B, S, C, P, R = 4, 96, 8, 16, 2
KS = 2 * R + 1
NCORES = 8
ZB = S // NCORES          # 12 output z per core
ZW = ZB + 2 * R           # 16 input z window
YT = 4                    # y-tile output size
YW = YT + 2 * R           # 8 input y window
NYT = S // YT             # 24 y tiles
XP = S + 2 * R            # 100 padded x
NX = B * S                # 384 matmul columns
FREE = B * XP * C         # 3200 free elems per conv-tile partition
OFREE = C * B * S         # 3072 out free elems (channel-major)

_cache = {}
LAST_RESULT = None


# ---------------------------------------------------------------- legalize

def _legalize_bir_json(bj):
    bir = json.loads(bj)
    n = 0
    for fn in bir.get("functions", []):
        for blk in fn.get("blocks", []):
            out = []
            for inst in blk.get("instructions", []):
                si = inst.get("sync_info")
                waits = (si or {}).get("on_wait") or []
                if len(waits) > 1:
                    for w in waits[:-1]:
                        n += 1
                        ev = {
                            "engine": inst.get("engine"),
                            "ins": [],
                            "name": f"I-waitfix-{n}",
                            "opcode": "EventSemaphore",
                            "outs": [],
                            "sync_info": {"on_update": [], "on_wait": [w]},
                        }
                        if "debug" in inst:
                            ev["debug"] = inst["debug"]
                        out.append(ev)
                    si["on_wait"] = waits[-1:]
                out.append(inst)
            blk["instructions"] = out
    return json.dumps(bir).encode()


def _attach_legalizer(nc):
    orig = nc.to_json_bytes

    def patched():
        return _legalize_bir_json(orig())

    nc.to_json_bytes = patched
    return nc


# ---------------------------------------------------------------- program

def _build_groups(c0):
    groups = []  # (channel, [pair indices])
    for c in range(C):
        ps = [p for p in range(P) if c0[p] == c]
        for i in range(0, len(ps), 2):
            groups.append((c, ps[i:i + 2]))
    return groups


def _build_program(c0, c1, A, iters=1):
    groups = _build_groups(c0)
    ngrp = len(groups)
    wcols = []
    off = 0
    for _, plist in groups:
        mg = 48 + 64 * (len(plist) - 1)
        wcols.append(off)
        off += KS * mg
    WCOLS = off

    nc = bass.Bass()
    slab = nc.dram_tensor("slab", [NYT, 128, FREE], BF16, kind="ExternalInput")
    inner = nc.dram_tensor("inner", [NYT, 48, OFREE], BF16, kind="ExternalInput")
    wts = nc.dram_tensor("wts", [128, WCOLS], BF16, kind="ExternalInput")
    gp = nc.dram_tensor("gp", [112, ngrp], F32, kind="ExternalInput")
    out = nc.dram_tensor("out", [NYT, 48, OFREE], F32, kind="ExternalOutput")

    with TileContext(nc) as tc, ExitStack() as ctx:
        wpool = ctx.enter_context(tc.tile_pool(name="wpool", bufs=1))
        wtile = wpool.tile([128, WCOLS], BF16)
        nc.gpsimd.dma_start(wtile[:], wts[:, :])
        gtile = wpool.tile([112, ngrp], F32)
        nc.gpsimd.dma_start(gtile[:], gp[:, :])

        # warm persistent-tensor deps once per consumer engine
        wps_pool = ctx.enter_context(tc.tile_pool(name="wps", bufs=1, space="PSUM"))
        wps = wps_pool.tile([1, 1], F32)
        nc.tensor.matmul(wps[:], wtile[:1, :1], wtile[:1, :1], start=True, stop=True)
        wact = wpool.tile([1, 1], F32)
        nc.scalar.copy(wact[:1, :1], gtile[:1, :1])

        cv_pool = ctx.enter_context(tc.tile_pool(name="cv", bufs=3))
        ot_pool = ctx.enter_context(tc.tile_pool(name="ot", bufs=3))
        ost_pool = ctx.enter_context(tc.tile_pool(name="ost", bufs=2))
        ps_pool = ctx.enter_context(tc.tile_pool(name="ps", bufs=6, space="PSUM"))
        pp_pool = ctx.enter_context(tc.tile_pool(name="pp", bufs=1, space="PSUM"))
        sq_pool = ctx.enter_context(tc.tile_pool(name="sq", bufs=2))
        ex_pool = ctx.enter_context(tc.tile_pool(name="ex", bufs=2))
        cp_pool = ctx.enter_context(tc.tile_pool(name="cp", bufs=2))
        pr_pool = ctx.enter_context(tc.tile_pool(name="pr", bufs=2))

        ndbl = sum(1 for _, pl in groups if len(pl) == 2)

        for yt in [t for _ in range(iters) for t in range(NYT)]:
            y0 = yt * YT
            cv = cv_pool.tile([128, FREE], BF16, tag="cv")
            nc.gpsimd.dma_start(cv[:], slab[yt])

            ot = ot_pool.tile([48, OFREE], BF16, tag="ot")
            nc.gpsimd.dma_start(ot[:], inner[yt])

            if yt > 0:
                # PE probe: absorb ACT psum-WAR deps from previous tile
                ppx = pp_pool.tile([1, 1], F32, tag="ppx")
                nc.tensor.matmul(ppx[:], wtile[:1, :1], wtile[:1, :1],
                                 start=True, stop=True)

            cvv = cv[:].rearrange("p (b x c) -> p b x c", b=B, x=XP, c=C)
            sq_all = sq_pool.tile([112, ngrp * NX], BF16, tag="sq")
            for g, (c, plist) in enumerate(groups):
                mg = 48 + 64 * (len(plist) - 1)
                ps = ps_pool.tile([112, NX], F32, tag="ps")
                for dx in range(KS):
                    lhsT = wtile[:, wcols[g] + dx * mg: wcols[g] + (dx + 1) * mg]
                    rhs = cvv[:, :, dx:dx + S, c]
                    nc.tensor.matmul(ps[:mg, :], lhsT, rhs,
                                     start=(dx == 0), stop=(dx == KS - 1))
                nc.scalar.activation(sq_all[:mg, g * NX:(g + 1) * NX], ps[:mg, :],
                                     mybir.ActivationFunctionType.Square,
                                     bias=gtile[:mg, g:g + 1], scale=1.0)

            ex_all = ex_pool.tile([112, ngrp * NX], BF16, tag="ex")
            nc.scalar.activation(ex_all[:], sq_all[:],
                                 mybir.ActivationFunctionType.Exp, scale=-1.0)

            # DVE probe: absorb ot-DMA dep before the add chain
            prb = pr_pool.tile([1, 1], BF16, tag="prb")
            nc.vector.tensor_copy(prb[:1, 0:1], ot[:1, :1])

            # partition down-shift copies for second-slot pairs
            excp = cp_pool.tile([48, max(ndbl, 1) * NX], BF16, tag="excp")
            kdbl = 0
            srcs = {}
            for g, (c, plist) in enumerate(groups):
                srcs[(g, 0)] = ex_all[0:48, g * NX:(g + 1) * NX]
                if len(plist) == 2:
                    dstc = excp[:, kdbl * NX:(kdbl + 1) * NX]
                    nc.vector.tensor_copy(dstc, ex_all[64:112, g * NX:(g + 1) * NX])
                    srcs[(g, 1)] = dstc
                    kdbl += 1

            otv = ot[:].rearrange("p (c f) -> p c f", c=C)
            for g, (c, plist) in enumerate(groups):
                for j, p in enumerate(plist):
                    sl = otv[:, int(c1[p]), :]
                    nc.vector.scalar_tensor_tensor(
                        sl, srcs[(g, j)], float(A[p]), sl,
                        op0=mybir.AluOpType.mult, op1=mybir.AluOpType.add)

            ost = ost_pool.tile([48, OFREE], F32, tag="ost")
            nc.vector.tensor_scalar(ost[:], ot[:], 1.0, 0.0,
                                    op0=mybir.AluOpType.min,
                                    op1=mybir.AluOpType.max)
            nc.gpsimd.dma_start(out[yt], ost[:])

    _attach_legalizer(nc)
    return nc, groups, wcols, WCOLS, ngrp


def _host_prep(kernels, m, s, h, T, groups, wcols, WCOLS):
    ngrp = len(groups)
    Tv = float(T[0])
    k = 1.0 / (np.sqrt(2.0) * s.astype(np.float64))
    wk = (kernels[..., 0, :].astype(np.float64) * k).astype(np.float32)
    wts = np.zeros((128, WCOLS), dtype=np.float32)
    gpa = np.zeros((112, ngrp), dtype=np.float32)
    for g, (c, plist) in enumerate(groups):
        mg = 48 + 64 * (len(plist) - 1)
        for j, p in enumerate(plist):
            gpa[j * 64:j * 64 + 48, g] = -m[p] * k[p]
        for dx in range(KS):
            Wm = np.zeros((128, mg), dtype=np.float32)
            for j, p in enumerate(plist):
                for dz in range(KS):
                    for dy in range(KS):
                        w = wk[dz, dy, dx, p]
                        for zo in range(ZB):
                            Wm[(zo + dz) * YW + dy: (zo + dz) * YW + dy + YT,
                               j * 64 + zo * YT: j * 64 + (zo + 1) * YT] += \
                                np.eye(YT, dtype=np.float32) * w
            wts[:, wcols[g] + dx * mg: wcols[g] + (dx + 1) * mg] = Wm
    return wts.astype(BF16_NP), gpa


def kernel(**inputs):
    global LAST_RESULT
    inp = np.ascontiguousarray(inputs["input"], dtype=np.float32)
    kernels = np.asarray(inputs["kernels"], dtype=np.float32)
    m = np.asarray(inputs["m"], dtype=np.float32)
    s = np.asarray(inputs["s"], dtype=np.float32)
    h = np.asarray(inputs["h"], dtype=np.float32)
    T = np.asarray(inputs["T"], dtype=np.float32)
    c0 = tuple(int(v) for v in inputs["c0_idx"])
    c1 = tuple(int(v) for v in inputs["c1_idx"])
    Tv = float(T[0])
    A = (2.0 * h / Tv).astype(np.float32)

    try:
        if BF16_NP is None:
            raise RuntimeError("ml_dtypes unavailable")
        iters = int(os.environ.get("KERNEL_ITERS", "1"))
        key = (c0, c1, A.tobytes(), iters)
        if key not in _cache:
            _cache[key] = _build_program(c0, c1, A, iters)
        nc, groups, wcols, WCOLS, ngrp = _cache[key]

        wts, gpa = _host_prep(kernels, m, s, h, T, groups, wcols, WCOLS)

        Cc = np.zeros(C, dtype=np.float32)
        for p in range(P):
            Cc[c1[p]] += -h[p] / Tv

        inp_b = inp.astype(BF16_NP)
        in_maps = []
        for kcore in range(NCORES):
            zidx = (np.arange(ZW) + ZB * kcore - R) % S
            slab = inp_b[:, zidx]
            slab = np.pad(slab, ((0, 0), (0, 0), (R, R), (R, R), (0, 0)),
                          mode="wrap")
            slab = slab.transpose(1, 2, 0, 3, 4)  # [ZW, 100, B, XP, C]
            slab2 = np.empty((NYT, 128, FREE), dtype=BF16_NP)
            for t in range(NYT):
                slab2[t] = slab[:, t * YT:t * YT + YW].reshape(128, FREE)
            # inner: [z, y, c, b, x] channel-major, input + Cc
            innr = (inp[:, ZB * kcore:ZB * (kcore + 1)] + Cc).astype(BF16_NP)
            innr = np.ascontiguousarray(innr.transpose(1, 2, 4, 0, 3)
                                        ).reshape(ZB, NYT, YT, OFREE)
            innr = np.ascontiguousarray(innr.transpose(1, 0, 2, 3)
                                        ).reshape(NYT, 48, OFREE)
            in_maps.append({"slab": slab2, "inner": innr,
                            "wts": wts, "gp": gpa})

        from concourse.bass_utils import run_bass_kernel_spmd
        trace = bool(int(os.environ.get("KERNEL_TRACE", "0")))
        res = run_bass_kernel_spmd(nc, in_maps, core_ids=list(range(NCORES)),
                                   trace=trace)
        LAST_RESULT = res
        parts = []
        for r in res.results:
            o = r["out"].reshape(NYT, ZB, YT, C, B, S)
            o = o.transpose(1, 0, 2, 3, 4, 5).reshape(ZB, S, C, B, S)
            parts.append(o.transpose(3, 0, 1, 4, 2))
        return np.ascontiguousarray(np.concatenate(parts, axis=1))
    except Exception:
        if os.environ.get("KERNEL_NO_FALLBACK"):
            raise
        return _numpy_fallback(inp, kernels, m, s, h, T,
                               np.array(c0), np.array(c1))


def _numpy_fallback(inp, kernels, m, s, h, T, c0, c1):
    xg = inp[..., c0]
    pad = np.pad(xg, ((0, 0), (R, R), (R, R), (R, R), (0, 0)), mode="wrap")
    pot = np.zeros_like(xg)
    for dz in range(KS):
        for dy in range(KS):
            for dx in range(KS):
                w = kernels[dz, dy, dx, 0, :]
                pot += w * pad[:, dz:dz + S, dy:dy + S, dx:dx + S, :]
    delta = ((np.exp(-(pot - m) ** 2 / (2.0 * s ** 2)) * 2.0 - 1.0) * h
             ).astype(np.float32)
    field = np.zeros_like(inp)
    for p in range(P):
        field[..., int(c1[p])] += delta[..., p]
    out = np.clip(inp + field / float(T[0]), 0.0, 1.0).astype(np.float32)
    return out
